# revision 1
# baseline (speedup 1.0000x reference)
"""GBST embedding kernel for Trainium2, data-parallel over batch on 8 cores.

Strategy per core (one batch element, [d_chunk, l] layout, 4 chunks of 128):
- Embedding gather is folded into the conv: y[do,l] = sum_k sum_v G_k[v,do] *
  onehot[v, l+k-2] with G_k = emb @ conv_w[:,:,k].T precomputed on host (bf16).
  Contraction over vocab (256 = 2 chunks) instead of d_in (512) halves PE work.
- Onehot built on device: ids broadcast via replicate-DMA + is_equal vs iota.
- Scores: s1 = score_w.T @ y on PE; block-pool sums for b=2,3,4 as strided adds.
- Softmax over the 4 upsampled scores in an l-major [128, 64] layout; softmax
  weights folded with 0.5/b (mean-pool scale + final downsample-by-2 scale) and
  collapsed onto the output t-grid (b=2,4 fully; b=3 onto a u-grid of 3-blocks).
- Weighted combine on DVE with bf16 muls and f32 accumulation; block pooling
  recomputed per segment to keep SBUF small; weight rows staged via DRAM and
  broadcast back with replicate DMAs. PE transposes [d, t] -> [t, d] for
  contiguous stores.
"""
import sys
sys.path.insert(0, "/opt/trn_rl_repo")
import numpy as np
import ml_dtypes

import concourse.bass as bass
import concourse.bacc as bacc
import concourse.tile as tile
from concourse import mybir
from concourse.bass_utils import run_bass_kernel_spmd

bf16 = ml_dtypes.bfloat16
F32 = mybir.dt.float32
BF = mybir.dt.bfloat16
OP = mybir.AluOpType

L, T, V, D, K = 8192, 4096, 256, 512, 5
NDC, NVC, NLT, LTS = 4, 2, 16, 512
N3 = 2731          # ceil(L/3)
TSEG = 1024        # combine segment width in t

TRACE = False
LAST_RESULT = None
_NC = None


def _build():
    nc = bacc.Bacc("TRN2", target_bir_lowering=False)
    ids_d = nc.dram_tensor("ids", [1, L], BF, kind="ExternalInput")
    gws_d = nc.dram_tensor("gws", [128, 40 * 128], BF, kind="ExternalInput")
    iot_d = nc.dram_tensor("iot", [128, 2], F32, kind="ExternalInput")
    scw_d = nc.dram_tensor("scw", [128, 4], BF, kind="ExternalInput")
    bias_d = nc.dram_tensor("bias", [128, 4], F32, kind="ExternalInput")
    ident_d = nc.dram_tensor("ident", [128, 128], F32, kind="ExternalInput")
    out_d = nc.dram_tensor("out", [T, D], F32, kind="ExternalOutput")
    # DRAM staging for broadcast-source weight rows
    w1erow_d = nc.dram_tensor("w1erow_d", [1, T], BF)
    w1orow_d = nc.dram_tensor("w1orow_d", [1, T], BF)
    cw2row_d = nc.dram_tensor("cw2row_d", [1, T], BF)
    cw4row_d = nc.dram_tensor("cw4row_d", [1, T], BF)
    cwarow_d = nc.dram_tensor("cwarow_d", [1, 1366], BF)
    cwbrow_d = nc.dram_tensor("cwbrow_d", [1, 1366], BF)
    cwcrow_d = nc.dram_tensor("cwcrow_d", [1, 1366], BF)
    cwdrow_d = nc.dram_tensor("cwdrow_d", [1, 1366], BF)

    with tile.TileContext(nc) as tc:
        with tc.tile_pool(name="const", bufs=1) as cst, \
             tc.tile_pool(name="persist", bufs=1) as per, \
             tc.tile_pool(name="rows", bufs=1) as rws, \
             tc.tile_pool(name="rowbig", bufs=1) as rwb, \
             tc.tile_pool(name="sm", bufs=1) as sm, \
             tc.tile_pool(name="ohp", bufs=2) as ohp, \
             tc.tile_pool(name="wseg", bufs=1) as wsg, \
             tc.tile_pool(name="segpool", bufs=2) as sgp, \
             tc.tile_pool(name="accp", bufs=1) as accp, \
             tc.tile_pool(name="ctp", bufs=3) as ctp, \
             tc.tile_pool(name="btp", bufs=2) as btp, \
             tc.tile_pool(name="otp", bufs=2) as otp, \
             tc.tile_pool(name="psA", bufs=3, space="PSUM") as psA, \
             tc.tile_pool(name="psB", bufs=2, space="PSUM") as psB, \
             tc.tile_pool(name="psT", bufs=2, space="PSUM") as psT:

            # ---- constants
            gws_t = cst.tile([128, 40 * 128], BF)
            nc.sync.dma_start(out=gws_t[:], in_=gws_d[:])
            iot_t = cst.tile([128, 2], F32)
            nc.sync.dma_start(out=iot_t[:], in_=iot_d[:])
            scw_t = cst.tile([128, 4], BF)
            nc.sync.dma_start(out=scw_t[:], in_=scw_d[:])
            bias_t = cst.tile([128, 4], F32)
            nc.sync.dma_start(out=bias_t[:], in_=bias_d[:])
            ident_t = cst.tile([128, 128], F32)
            nc.sync.dma_start(out=ident_t[:], in_=ident_d[:])

            # ---- persistent tensors
            y = [per.tile([128, L + 4], BF, name=f"y{dc}", tag=f"y{dc}")
                 for dc in range(NDC)]
            s1row = rws.tile([1, L + 4], F32)
            s3row = rws.tile([1, N3], F32)
            cwarow = rws.tile([1, 1366], BF)
            cwbrow = rws.tile([1, 1366], BF)
            cwcrow = rws.tile([1, 1366], BF)
            cwdrow = rws.tile([1, 1366], BF)

            for dc in range(NDC):
                nc.vector.memset(y[dc][:, L:L + 4], 0.0)
            nc.vector.memset(s1row[0:1, L:L + 4], 0.0)

            # ---- conv + gather + s1, per l-tile
            for i in range(NLT):
                c0 = i * LTS - 2
                c1 = i * LTS + 514
                lo = max(c0, 0)
                hi = min(c1, L)
                d0 = lo - c0          # dst col where valid data starts
                d1 = 516 - (c1 - hi)  # dst col where valid data ends
                idst = ohp.tile([128, 516], BF, tag="idst")
                nc.sync.dma_start(out=idst[:, d0:d1],
                                  in_=ids_d[0:1, lo:hi].partition_broadcast(128))
                ohs = []
                for vc in range(NVC):
                    oh = ohp.tile([128, 516], BF, tag=f"oh{vc}", name=f"oh{vc}_{i}")
                    if d0 > 0:
                        nc.vector.memset(oh[:, 0:d0], 0.0)
                    if d1 < 516:
                        nc.vector.memset(oh[:, d1:516], 0.0)
                    nc.vector.tensor_scalar(out=oh[:, d0:d1], in0=idst[:, d0:d1],
                                            scalar1=iot_t[:, vc:vc + 1], scalar2=None,
                                            op0=OP.is_equal)
                    ohs.append(oh)
                for dc in range(NDC):
                    ps = psA.tile([128, LTS], F32, tag="convps", name=f"ps_{i}_{dc}")
                    for j in range(10):
                        k, vc = divmod(j, 2)
                        nc.tensor.matmul(
                            out=ps[:],
                            lhsT=gws_t[:, ((k * 2 + vc) * 4 + dc) * 128:
                                       ((k * 2 + vc) * 4 + dc) * 128 + 128],
                            rhs=ohs[vc][:, k:k + LTS],
                            start=(j == 0), stop=(j == 9))
                    nc.scalar.activation(out=y[dc][:, i * LTS:(i + 1) * LTS], in_=ps[:],
                                         func=mybir.ActivationFunctionType.Identity,
                                         bias=bias_t[:, dc:dc + 1])
                ps1 = psB.tile([1, LTS], F32, tag="s1ps", name=f"ps1_{i}")
                for dc in range(NDC):
                    nc.tensor.matmul(out=ps1[:], lhsT=scw_t[:, dc:dc + 1],
                                     rhs=y[dc][:, i * LTS:(i + 1) * LTS],
                                     start=(dc == 0), stop=(dc == NDC - 1))
                nc.scalar.copy(out=s1row[0:1, i * LTS:(i + 1) * LTS], in_=ps1[:])

            # ---- score pooling + softmax in l-major [128, 64] layout
            S = sm.tile([128, 256], F32)
            nc.sync.dma_start(out=S[:, 0:64], in_=s1row[0:1, 0:L])
            s2r = sm.tile([128, 32], F32)
            Spair = S[:, 0:64].rearrange("p (n two) -> p n two", two=2)
            nc.vector.tensor_tensor(out=s2r[:], in0=Spair[:, :, 0],
                                    in1=Spair[:, :, 1], op=OP.add)
            s4r = sm.tile([128, 16], F32)
            s2pair = s2r[:].rearrange("p (n two) -> p n two", two=2)
            nc.vector.tensor_tensor(out=s4r[:], in0=s2pair[:, :, 0],
                                    in1=s2pair[:, :, 1], op=OP.add)
            nc.vector.tensor_scalar(
                out=S[:, 64:128].rearrange("p (n two) -> p n two", two=2),
                in0=s2r[:].unsqueeze(2).to_broadcast([128, 32, 2]),
                scalar1=0.5, scalar2=None, op0=OP.mult)
            nc.vector.tensor_scalar(
                out=S[:, 192:256].rearrange("p (n four) -> p n four", four=4),
                in0=s4r[:].unsqueeze(2).to_broadcast([128, 16, 4]),
                scalar1=0.25, scalar2=None, op0=OP.mult)
            nc.vector.tensor_tensor(out=s3row[0:1, :], in0=s1row[0:1, 0:3 * N3:3],
                                    in1=s1row[0:1, 1:3 * N3 + 1:3], op=OP.add)
            nc.vector.tensor_tensor(out=s3row[0:1, :], in0=s3row[0:1, :],
                                    in1=s1row[0:1, 2:3 * N3 + 2:3], op=OP.add)
            us3row = rwb.tile([1, 3 * N3], F32, tag="rowbig")
            nc.vector.tensor_copy(
                out=us3row[0:1, :],
                in_=s3row[0:1, :].unsqueeze(2).to_broadcast([1, N3, 3]))
            nc.sync.dma_start(out=S[:, 128:192], in_=us3row[0:1, 0:L])
            nc.vector.tensor_scalar(out=S[:, 128:192], in0=S[:, 128:192],
                                    scalar1=1.0 / 3.0, scalar2=None, op0=OP.mult)

            mM = sm.tile([128, 64], F32)
            nc.vector.tensor_tensor(out=mM[:], in0=S[:, 0:64], in1=S[:, 64:128],
                                    op=OP.max)
            nc.vector.tensor_tensor(out=mM[:], in0=mM[:], in1=S[:, 128:192], op=OP.max)
            nc.vector.tensor_tensor(out=mM[:], in0=mM[:], in1=S[:, 192:256], op=OP.max)
            S4v = S[:].rearrange("p (four n) -> p four n", four=4)
            nc.vector.tensor_tensor(out=S4v, in0=S4v,
                                    in1=mM[:].unsqueeze(1).to_broadcast([128, 4, 64]),
                                    op=OP.subtract)
            nc.scalar.activation(out=S[:], in_=S[:],
                                 func=mybir.ActivationFunctionType.Exp)
            Z = sm.tile([128, 64], F32)
            nc.vector.tensor_tensor(out=Z[:], in0=S[:, 0:64], in1=S[:, 64:128],
                                    op=OP.add)
            nc.vector.tensor_tensor(out=Z[:], in0=Z[:], in1=S[:, 128:192], op=OP.add)
            nc.vector.tensor_tensor(out=Z[:], in0=Z[:], in1=S[:, 192:256], op=OP.add)
            R = sm.tile([128, 64], F32)
            nc.vector.reciprocal(out=R[:], in_=Z[:])
            W = sm.tile([128, 256], F32)
            W4v = W[:].rearrange("p (four n) -> p four n", four=4)
            nc.vector.tensor_tensor(out=W4v, in0=S4v,
                                    in1=R[:].unsqueeze(1).to_broadcast([128, 4, 64]),
                                    op=OP.mult)
            # weight extraction, 0.5/b folded
            W1e = sm.tile([128, 32], BF)
            W1o = sm.tile([128, 32], BF)
            W1pair = W[:, 0:64].rearrange("p (n two) -> p n two", two=2)
            nc.vector.tensor_scalar(out=W1e[:], in0=W1pair[:, :, 0], scalar1=0.5,
                                    scalar2=None, op0=OP.mult)
            nc.vector.tensor_scalar(out=W1o[:], in0=W1pair[:, :, 1], scalar1=0.5,
                                    scalar2=None, op0=OP.mult)
            tmp32 = sm.tile([128, 32], F32)
            W2pair = W[:, 64:128].rearrange("p (n two) -> p n two", two=2)
            nc.vector.tensor_tensor(out=tmp32[:], in0=W2pair[:, :, 0],
                                    in1=W2pair[:, :, 1], op=OP.add)
            CW2 = sm.tile([128, 32], BF)
            nc.vector.tensor_scalar(out=CW2[:], in0=tmp32[:], scalar1=0.25,
                                    scalar2=None, op0=OP.mult)
            tmp32b = sm.tile([128, 32], F32)
            W4pair = W[:, 192:256].rearrange("p (n two) -> p n two", two=2)
            nc.vector.tensor_tensor(out=tmp32b[:], in0=W4pair[:, :, 0],
                                    in1=W4pair[:, :, 1], op=OP.add)
            CW4 = sm.tile([128, 32], BF)
            nc.vector.tensor_scalar(out=CW4[:], in0=tmp32b[:], scalar1=0.125,
                                    scalar2=None, op0=OP.mult)
            W3 = sm.tile([128, 64], BF)
            nc.vector.tensor_scalar(out=W3[:], in0=W[:, 128:192], scalar1=1.0 / 6.0,
                                    scalar2=None, op0=OP.mult)
            # rows: reshape DMAs to DRAM staging; b3 u-grid rows via w3row
            nc.sync.dma_start(out=w1erow_d[0:1, :], in_=W1e[:])
            nc.sync.dma_start(out=w1orow_d[0:1, :], in_=W1o[:])
            nc.sync.dma_start(out=cw2row_d[0:1, :], in_=CW2[:])
            nc.sync.dma_start(out=cw4row_d[0:1, :], in_=CW4[:])
            w3row = rwb.tile([1, L + 10], BF, tag="rowbig")
            nc.vector.memset(w3row[0:1, L:L + 10], 0.0)
            nc.sync.dma_start(out=w3row[0:1, 0:L], in_=W3[:])
            nc.vector.tensor_tensor(out=cwarow[0:1, :], in0=w3row[0:1, 0:8196:6],
                                    in1=w3row[0:1, 1:8197:6], op=OP.add)
            nc.vector.tensor_copy(out=cwbrow[0:1, :], in_=w3row[0:1, 2:8198:6])
            nc.vector.tensor_copy(out=cwcrow[0:1, :], in_=w3row[0:1, 3:8199:6])
            nc.vector.tensor_tensor(out=cwdrow[0:1, :], in0=w3row[0:1, 4:8200:6],
                                    in1=w3row[0:1, 5:8201:6], op=OP.add)
            nc.sync.dma_start(out=cwarow_d[:], in_=cwarow[:])
            nc.sync.dma_start(out=cwbrow_d[:], in_=cwbrow[:])
            nc.sync.dma_start(out=cwcrow_d[:], in_=cwcrow[:])
            nc.sync.dma_start(out=cwdrow_d[:], in_=cwdrow[:])

            # ---- combine + transpose + store, segmented over t
            ov = out_d[:].rearrange("(tb p) (dc c) -> p tb dc c", p=128, c=128)
            for s in range(T // TSEG):
                t0 = s * TSEG
                # u-grid windows for the three b=3 residue classes
                tA0 = t0 + (-t0) % 3
                nA = len(range(tA0, t0 + TSEG, 3))
                uA0 = tA0 // 3
                tB0 = t0 + (1 - t0) % 3
                nB = len(range(tB0, t0 + TSEG, 3))
                uB0 = (tB0 - 1) // 3
                tD0 = t0 + (2 - t0) % 3
                nD = len(range(tD0, t0 + TSEG, 3))
                uD0 = (tD0 - 2) // 3
                jbase = min(2 * uA0, 2 * uB0, 2 * uD0 + 1)
                jend = max(2 * (uA0 + nA - 1), 2 * (uB0 + nB - 1) + 1,
                           2 * (uD0 + nD - 1) + 1)
                nJ = jend - jbase + 1

                w1e_s = wsg.tile([128, TSEG], BF, tag="w1e", name=f"w1e_{s}")
                nc.sync.dma_start(
                    out=w1e_s[:],
                    in_=w1erow_d[0:1, t0:t0 + TSEG].partition_broadcast(128))
                w1o_s = wsg.tile([128, TSEG], BF, tag="w1o", name=f"w1o_{s}")
                nc.sync.dma_start(
                    out=w1o_s[:],
                    in_=w1orow_d[0:1, t0:t0 + TSEG].partition_broadcast(128))
                cw2_s = wsg.tile([128, TSEG], BF, tag="cw2", name=f"cw2_{s}")
                nc.sync.dma_start(
                    out=cw2_s[:],
                    in_=cw2row_d[0:1, t0:t0 + TSEG].partition_broadcast(128))
                cw4_s = wsg.tile([128, TSEG], BF, tag="cw4", name=f"cw4_{s}")
                nc.sync.dma_start(
                    out=cw4_s[:],
                    in_=cw4row_d[0:1, t0:t0 + TSEG].partition_broadcast(128))
                cwa_s = wsg.tile([128, 342], BF, tag="cwa", name=f"cwa_{s}")
                nc.sync.dma_start(
                    out=cwa_s[:, 0:nA],
                    in_=cwarow_d[0:1, uA0:uA0 + nA].partition_broadcast(128))
                cwb_s = wsg.tile([128, 342], BF, tag="cwb", name=f"cwb_{s}")
                nc.sync.dma_start(
                    out=cwb_s[:, 0:nB],
                    in_=cwbrow_d[0:1, uB0:uB0 + nB].partition_broadcast(128))
                cwc_s = wsg.tile([128, 342], BF, tag="cwc", name=f"cwc_{s}")
                nc.sync.dma_start(
                    out=cwc_s[:, 0:nB],
                    in_=cwcrow_d[0:1, uB0:uB0 + nB].partition_broadcast(128))
                cwd_s = wsg.tile([128, 342], BF, tag="cwd", name=f"cwd_{s}")
                nc.sync.dma_start(
                    out=cwd_s[:, 0:nD],
                    in_=cwdrow_d[0:1, uD0:uD0 + nD].partition_broadcast(128))

                for dc in range(NDC):
                    ypair = y[dc][:, 0:L].rearrange("p (t two) -> p t two", two=2)
                    # per-seg pooled blocks
                    p2_s = sgp.tile([128, TSEG], BF, tag="p2s", name=f"p2s_{s}_{dc}")
                    nc.vector.tensor_tensor(out=p2_s[:], in0=ypair[:, t0:t0 + TSEG, 0],
                                            in1=ypair[:, t0:t0 + TSEG, 1], op=OP.add)
                    p4_s = sgp.tile([128, TSEG // 2], BF, tag="p4s",
                                    name=f"p4s_{s}_{dc}")
                    p2sp = p2_s[:].rearrange("p (v two) -> p v two", two=2)
                    nc.vector.tensor_tensor(out=p4_s[:], in0=p2sp[:, :, 0],
                                            in1=p2sp[:, :, 1], op=OP.add)
                    p3_s = sgp.tile([128, 688], BF, tag="p3s", name=f"p3s_{s}_{dc}")
                    nc.vector.tensor_tensor(
                        out=p3_s[:, 0:nJ],
                        in0=y[dc][:, 3 * jbase:3 * (jbase + nJ) - 2:3],
                        in1=y[dc][:, 3 * jbase + 1:3 * (jbase + nJ) - 1:3], op=OP.add)
                    nc.vector.tensor_tensor(
                        out=p3_s[:, 0:nJ], in0=p3_s[:, 0:nJ],
                        in1=y[dc][:, 3 * jbase + 2:3 * (jbase + nJ):3], op=OP.add)

                    m1e = ctp.tile([128, TSEG], BF, tag="ct", name=f"m1e_{s}_{dc}")
                    nc.vector.tensor_tensor(out=m1e[:], in0=ypair[:, t0:t0 + TSEG, 0],
                                            in1=w1e_s[:], op=OP.mult)
                    m1o = ctp.tile([128, TSEG], BF, tag="ct", name=f"m1o_{s}_{dc}")
                    nc.vector.tensor_tensor(out=m1o[:], in0=ypair[:, t0:t0 + TSEG, 1],
                                            in1=w1o_s[:], op=OP.mult)
                    a12 = ctp.tile([128, TSEG], BF, tag="ct", name=f"a12_{s}_{dc}")
                    nc.vector.tensor_tensor(out=a12[:], in0=m1e[:], in1=m1o[:],
                                            op=OP.add)
                    m2 = ctp.tile([128, TSEG], BF, tag="ct", name=f"m2_{s}_{dc}")
                    nc.vector.tensor_tensor(out=m2[:], in0=p2_s[:],
                                            in1=cw2_s[:], op=OP.mult)
                    m4 = ctp.tile([128, TSEG], BF, tag="ct", name=f"m4_{s}_{dc}")
                    nc.vector.tensor_tensor(
                        out=m4[:],
                        in0=p4_s[:].unsqueeze(2).to_broadcast([128, TSEG // 2, 2]),
                        in1=cw4_s[:], op=OP.mult)
                    acc = accp.tile([128, TSEG], F32, tag="acc", name=f"acc_{s}_{dc}")
                    nc.vector.tensor_tensor(out=acc[:], in0=a12[:], in1=m2[:],
                                            op=OP.add)
                    nc.vector.tensor_tensor(out=acc[:], in0=acc[:], in1=m4[:],
                                            op=OP.add)
                    # b=3 terms on their u-grids
                    for bi, (n_u, u0, coff, poff, cw_s) in enumerate((
                            (nA, uA0, tA0 - t0, 0, cwa_s),
                            (nB, uB0, tB0 - t0, 0, cwb_s),
                            (nB, uB0, tB0 - t0, 1, cwc_s),
                            (nD, uD0, tD0 - t0, 1, cwd_s))):
                        tb3 = btp.tile([128, 342], BF, tag="bt",
                                       name=f"tb3_{s}_{dc}_{bi}")
                        j0 = 2 * u0 + poff - jbase
                        nc.vector.tensor_tensor(
                            out=tb3[:, 0:n_u],
                            in0=p3_s[:, j0:j0 + 2 * n_u - 1:2],
                            in1=cw_s[:, 0:n_u], op=OP.mult)
                        accv = acc[:, coff:TSEG:3]
                        nc.vector.tensor_tensor(out=accv[:, 0:n_u], in0=accv[:, 0:n_u],
                                                in1=tb3[:, 0:n_u], op=OP.add)
                    # transpose [d, t] -> [t, d] and store
                    for q4 in range(2):
                        pt = psT.tile([128, 512], F32, tag="tp",
                                      name=f"pt_{s}_{dc}_{q4}")
                        for q in range(4):
                            nc.tensor.transpose(
                                out=pt[:, q * 128:(q + 1) * 128],
                                in_=acc[:, (q4 * 4 + q) * 128:(q4 * 4 + q + 1) * 128],
                                identity=ident_t[:])
                        ot = otp.tile([128, 512], F32, tag="ot",
                                      name=f"ot_{s}_{dc}_{q4}")
                        nc.scalar.copy(out=ot[:], in_=pt[:])
                        tb0 = s * 8 + q4 * 4
                        nc.sync.dma_start(
                            out=ov[:, tb0:tb0 + 4, dc, :],
                            in_=ot[:].rearrange("p (tb c) -> p tb c", c=128))
    nc.compile()
    return nc


def _get_nc():
    global _NC
    if _NC is None:
        _NC = _build()
    return _NC


def kernel(input_ids, emb, conv_w, conv_b, score_w):
    global LAST_RESULT
    nc = _get_nc()
    input_ids = np.asarray(input_ids)
    emb = np.asarray(emb, dtype=np.float32)
    conv_w = np.asarray(conv_w, dtype=np.float32)
    conv_b = np.asarray(conv_b, dtype=np.float32)
    score_w = np.asarray(score_w, dtype=np.float32)
    B = input_ids.shape[0]

    G = np.einsum("oik,vi->kvo", conv_w.astype(np.float64),
                  emb.astype(np.float64)).astype(np.float32)  # [K, V, D]
    gws = np.zeros((128, 40, 128), np.float32)
    for k in range(K):
        for vc in range(NVC):
            for dc in range(NDC):
                gws[:, (k * 2 + vc) * 4 + dc, :] = \
                    G[k, vc * 128:(vc + 1) * 128, dc * 128:(dc + 1) * 128]
    gws = gws.reshape(128, 40 * 128).astype(bf16)
    iot = np.stack([np.arange(128), np.arange(128) + 128], axis=1).astype(np.float32)
    scw = score_w.reshape(4, 128).T.astype(bf16)
    biasm = conv_b.reshape(4, 128).T.astype(np.float32)
    ident = np.eye(128, dtype=np.float32)
    idsb = input_ids.astype(np.float32).astype(bf16)

    in_maps = [{"ids": np.ascontiguousarray(idsb[c:c + 1]), "gws": gws, "iot": iot,
                "scw": scw, "bias": biasm, "ident": ident} for c in range(B)]
    res = run_bass_kernel_spmd(nc, in_maps, core_ids=list(range(B)), trace=TRACE)
    LAST_RESULT = res
    return np.stack([res.results[c]["out"] for c in range(B)]).astype(np.float32)



# revision 7
# speedup vs baseline: 1.1262x; 1.1262x over previous
"""GBST embedding kernel for Trainium2, data-parallel over batch on 8 cores.

Strategy per core (one batch element, [d_chunk, l] layout, 4 chunks of 128):
- Embedding gather folded into the conv: y[do,l] = sum_k sum_v G_k[v,do] *
  onehot[v, l+k-2] with G_k = emb @ conv_w[:,:,k].T precomputed on host (bf16).
- Onehot built ONCE for full L on DVE (is_equal vs iota), conv matmuls slide
  windows over it. Scores s1 = score_w.T @ y on PE per l-tile.
- Pipelined in 4 chunks of 2048 l / 1024 t: after the conv tiles a chunk
  needs are emitted, that chunk's softmax + weighted combine are emitted, so
  DVE combine work overlaps PE conv work of later chunks.
- Softmax per chunk in l-major [128, 16]-per-plane layout; weights folded
  with 0.5/b and collapsed onto the output t-grid (b=3 onto a u-grid).
- Combine: DVE bf16 muls/adds (bf16 accumulator); block pooling on GPSIMD;
  PE transposes [d, t] -> [t, d] for contiguous stores.
"""
import sys
sys.path.insert(0, "/opt/trn_rl_repo")
import numpy as np
import ml_dtypes

import concourse.bass as bass
import concourse.bacc as bacc
import concourse.tile as tile
from concourse import mybir
from concourse.bass_utils import run_bass_kernel_spmd

bf16 = ml_dtypes.bfloat16
F32 = mybir.dt.float32
BF = mybir.dt.bfloat16
OP = mybir.AluOpType

L, T, V, D, K = 8192, 4096, 256, 512, 5
NDC, NVC, NLT, LTS = 4, 2, 16, 512
TSEG = 1024        # chunk width in t (2048 in l)
NCH = 4

TRACE = False
LAST_RESULT = None
_NC = None


def _ceil_div(a, b):
    return -(-a // b)


def _build():
    nc = bacc.Bacc("TRN2", target_bir_lowering=False)
    ids_d = nc.dram_tensor("ids", [1, L], BF, kind="ExternalInput")
    gws_d = nc.dram_tensor("gws", [128, 40 * 128], BF, kind="ExternalInput")
    iot_d = nc.dram_tensor("iot", [128, 2], F32, kind="ExternalInput")
    scw_d = nc.dram_tensor("scw", [128, 4], BF, kind="ExternalInput")
    bias_d = nc.dram_tensor("bias", [128, 4], F32, kind="ExternalInput")
    ident_d = nc.dram_tensor("ident", [128, 128], BF, kind="ExternalInput")
    out_d = nc.dram_tensor("out", [T, D], F32, kind="ExternalOutput")
    # DRAM staging for broadcast-source weight rows
    w1erow_d = nc.dram_tensor("w1erow_d", [1, T], BF)
    w1orow_d = nc.dram_tensor("w1orow_d", [1, T], BF)
    cw2row_d = nc.dram_tensor("cw2row_d", [1, T], BF)
    cw4row_d = nc.dram_tensor("cw4row_d", [1, T], BF)
    cwarow_d = nc.dram_tensor("cwarow_d", [1, 1368], BF)
    cwbrow_d = nc.dram_tensor("cwbrow_d", [1, 1368], BF)
    cwcrow_d = nc.dram_tensor("cwcrow_d", [1, 1368], BF)
    cwdrow_d = nc.dram_tensor("cwdrow_d", [1, 1368], BF)

    with tile.TileContext(nc) as tc:
        with tc.tile_pool(name="const", bufs=1) as cst, \
             tc.tile_pool(name="persist", bufs=1) as per, \
             tc.tile_pool(name="rows", bufs=1) as rws, \
             tc.tile_pool(name="rowbig", bufs=1) as rwb, \
             tc.tile_pool(name="sm", bufs=1) as sm, \
             tc.tile_pool(name="wseg", bufs=1) as wsg, \
             tc.tile_pool(name="segpool", bufs=1) as sgp, \
             tc.tile_pool(name="accp", bufs=1) as accp, \
             tc.tile_pool(name="ctp", bufs=3) as ctp, \
             tc.tile_pool(name="btp", bufs=1) as btp, \
             tc.tile_pool(name="otp", bufs=1) as otp, \
             tc.tile_pool(name="psA", bufs=3, space="PSUM") as psA, \
             tc.tile_pool(name="psB", bufs=2, space="PSUM") as psB, \
             tc.tile_pool(name="psT", bufs=2, space="PSUM") as psT:

            # ---- constants
            gws_t = cst.tile([128, 40 * 128], BF)
            nc.sync.dma_start(out=gws_t[:], in_=gws_d[:])
            iot_t = cst.tile([128, 2], F32)
            nc.sync.dma_start(out=iot_t[:], in_=iot_d[:])
            scw_t = cst.tile([128, 4], BF)
            nc.sync.dma_start(out=scw_t[:], in_=scw_d[:])
            bias_t = cst.tile([128, 4], F32)
            nc.sync.dma_start(out=bias_t[:], in_=bias_d[:])
            ident_t = cst.tile([128, 128], BF)
            nc.sync.dma_start(out=ident_t[:], in_=ident_d[:])

            # ---- persistent tensors
            y = [per.tile([128, L + 4], BF, name=f"y{dc}", tag=f"y{dc}")
                 for dc in range(NDC)]
            s1row = rws.tile([1, L + 4], BF)
            w3row = rws.tile([1, L + 12], BF)
            for dc in range(NDC):
                nc.vector.memset(y[dc][:, L:L + 4], 0.0)
            nc.vector.memset(s1row[0:1, L:L + 4], 0.0)

            # ---- full-L onehot: col x = l + 2, x in [0, L+4)
            idstF = per.tile([128, L + 4], BF, name="idstF", tag="idstF")
            nc.vector.memset(idstF[:, 0:2], -7.0)
            nc.vector.memset(idstF[:, L + 2:L + 4], -7.0)
            nc.sync.dma_start(out=idstF[:, 2:L + 2],
                              in_=ids_d[0:1, 0:L].partition_broadcast(128))
            ohF = []
            for vc in range(NVC):
                oh = per.tile([128, L + 4], BF, name=f"ohF{vc}", tag=f"ohF{vc}")
                nc.vector.tensor_scalar(out=oh[:], in0=idstF[:],
                                        scalar1=iot_t[:, vc:vc + 1], scalar2=None,
                                        op0=OP.is_equal)
                ohF.append(oh)

            ov = out_d[:].rearrange("(tb p) (dc c) -> p tb dc c", p=128, c=128)
            chunk_after = {4: 0, 8: 1, 12: 2, 15: 3}

            for i in range(NLT):
                # ---- conv + gather + s1 for l-tile i
                for dc in range(NDC):
                    ps = psA.tile([128, LTS], F32, tag="convps", name=f"ps_{i}_{dc}")
                    for j in range(10):
                        k, vc = divmod(j, 2)
                        nc.tensor.matmul(
                            out=ps[:],
                            lhsT=gws_t[:, ((k * 2 + vc) * 4 + dc) * 128:
                                       ((k * 2 + vc) * 4 + dc) * 128 + 128],
                            rhs=ohF[vc][:, i * LTS + k:i * LTS + k + LTS],
                            start=(j == 0), stop=(j == 9))
                    nc.scalar.activation(out=y[dc][:, i * LTS:(i + 1) * LTS], in_=ps[:],
                                         func=mybir.ActivationFunctionType.Identity,
                                         bias=bias_t[:, dc:dc + 1])
                ps1 = psB.tile([1, LTS], F32, tag="s1ps", name=f"ps1_{i}")
                for dc in range(NDC):
                    nc.tensor.matmul(out=ps1[:], lhsT=scw_t[:, dc:dc + 1],
                                     rhs=y[dc][:, i * LTS:(i + 1) * LTS],
                                     start=(dc == 0), stop=(dc == NDC - 1))
                nc.scalar.copy(out=s1row[0:1, i * LTS:(i + 1) * LTS], in_=ps1[:])

                if i not in chunk_after:
                    continue
                c = chunk_after[i]
                t0 = c * TSEG
                t1 = t0 + TSEG
                l0 = 2 * t0

                # ---- scores + softmax for chunk c, l-major [128, 16] planes
                j0 = l0 // 3
                j1 = (l0 + 2047) // 3
                nj = j1 - j0 + 1
                off3 = l0 - 3 * j0
                s3c = rws.tile([1, 688], F32, tag="s3c", name=f"s3c_{c}")
                nc.vector.tensor_tensor(
                    out=s3c[0:1, 0:nj],
                    in0=s1row[0:1, 3 * j0:3 * j0 + 3 * nj:3],
                    in1=s1row[0:1, 3 * j0 + 1:3 * j0 + 1 + 3 * nj:3], op=OP.add)
                nc.vector.tensor_tensor(
                    out=s3c[0:1, 0:nj], in0=s3c[0:1, 0:nj],
                    in1=s1row[0:1, 3 * j0 + 2:3 * j0 + 2 + 3 * nj:3], op=OP.add)
                us3c = rwb.tile([1, 2064], F32, tag="us3", name=f"us3_{c}")
                nc.vector.tensor_copy(
                    out=us3c[0:1, 0:3 * nj],
                    in_=s3c[0:1, 0:nj].unsqueeze(2).to_broadcast([1, nj, 3]))

                S = sm.tile([128, 64], F32, tag="S", name=f"S_{c}")
                Sb1 = sm.tile([128, 16], BF, tag="Sb1", name=f"Sb1_{c}")
                nc.sync.dma_start(out=Sb1[:], in_=s1row[0:1, l0:l0 + 2048])
                nc.vector.tensor_copy(out=S[:, 0:16], in_=Sb1[:])
                nc.sync.dma_start(out=S[:, 32:48], in_=us3c[0:1, off3:off3 + 2048])
                s2r = sm.tile([128, 8], F32, tag="s2r", name=f"s2r_{c}")
                Spair = S[:, 0:16].rearrange("p (n two) -> p n two", two=2)
                nc.vector.tensor_tensor(out=s2r[:], in0=Spair[:, :, 0],
                                        in1=Spair[:, :, 1], op=OP.add)
                s4r = sm.tile([128, 4], F32, tag="s4r", name=f"s4r_{c}")
                s2pair = s2r[:].rearrange("p (n two) -> p n two", two=2)
                nc.vector.tensor_tensor(out=s4r[:], in0=s2pair[:, :, 0],
                                        in1=s2pair[:, :, 1], op=OP.add)
                nc.vector.tensor_scalar(
                    out=S[:, 16:32].rearrange("p (n two) -> p n two", two=2),
                    in0=s2r[:].unsqueeze(2).to_broadcast([128, 8, 2]),
                    scalar1=0.5, scalar2=None, op0=OP.mult)
                nc.vector.tensor_scalar(
                    out=S[:, 48:64].rearrange("p (n four) -> p n four", four=4),
                    in0=s4r[:].unsqueeze(2).to_broadcast([128, 4, 4]),
                    scalar1=0.25, scalar2=None, op0=OP.mult)
                nc.vector.tensor_scalar(out=S[:, 32:48], in0=S[:, 32:48],
                                        scalar1=1.0 / 3.0, scalar2=None, op0=OP.mult)

                mM = sm.tile([128, 16], F32, tag="mM", name=f"mM_{c}")
                nc.vector.tensor_tensor(out=mM[:], in0=S[:, 0:16], in1=S[:, 16:32],
                                        op=OP.max)
                nc.vector.tensor_tensor(out=mM[:], in0=mM[:], in1=S[:, 32:48],
                                        op=OP.max)
                nc.vector.tensor_tensor(out=mM[:], in0=mM[:], in1=S[:, 48:64],
                                        op=OP.max)
                S4v = S[:].rearrange("p (four n) -> p four n", four=4)
                nc.vector.tensor_tensor(
                    out=S4v, in0=S4v,
                    in1=mM[:].unsqueeze(1).to_broadcast([128, 4, 16]), op=OP.subtract)
                nc.scalar.activation(out=S[:], in_=S[:],
                                     func=mybir.ActivationFunctionType.Exp)
                Z = sm.tile([128, 16], F32, tag="Z", name=f"Z_{c}")
                nc.vector.tensor_tensor(out=Z[:], in0=S[:, 0:16], in1=S[:, 16:32],
                                        op=OP.add)
                nc.vector.tensor_tensor(out=Z[:], in0=Z[:], in1=S[:, 32:48], op=OP.add)
                nc.vector.tensor_tensor(out=Z[:], in0=Z[:], in1=S[:, 48:64], op=OP.add)
                R = sm.tile([128, 16], F32, tag="R", name=f"R_{c}")
                nc.vector.reciprocal(out=R[:], in_=Z[:])
                W = sm.tile([128, 64], F32, tag="W", name=f"W_{c}")
                W4v = W[:].rearrange("p (four n) -> p four n", four=4)
                nc.vector.tensor_tensor(
                    out=W4v, in0=S4v,
                    in1=R[:].unsqueeze(1).to_broadcast([128, 4, 16]), op=OP.mult)
                # weight extraction, 0.5/b folded
                W1e = sm.tile([128, 8], BF, tag="W1e", name=f"W1e_{c}")
                W1o = sm.tile([128, 8], BF, tag="W1o", name=f"W1o_{c}")
                W1pair = W[:, 0:16].rearrange("p (n two) -> p n two", two=2)
                nc.vector.tensor_scalar(out=W1e[:], in0=W1pair[:, :, 0], scalar1=0.5,
                                        scalar2=None, op0=OP.mult)
                nc.vector.tensor_scalar(out=W1o[:], in0=W1pair[:, :, 1], scalar1=0.5,
                                        scalar2=None, op0=OP.mult)
                tmp32 = sm.tile([128, 8], F32, tag="tmp32", name=f"tmp32_{c}")
                W2pair = W[:, 16:32].rearrange("p (n two) -> p n two", two=2)
                nc.vector.tensor_tensor(out=tmp32[:], in0=W2pair[:, :, 0],
                                        in1=W2pair[:, :, 1], op=OP.add)
                CW2 = sm.tile([128, 8], BF, tag="CW2", name=f"CW2_{c}")
                nc.vector.tensor_scalar(out=CW2[:], in0=tmp32[:], scalar1=0.25,
                                        scalar2=None, op0=OP.mult)
                tmp32b = sm.tile([128, 8], F32, tag="tmp32b", name=f"tmp32b_{c}")
                W4pair = W[:, 48:64].rearrange("p (n two) -> p n two", two=2)
                nc.vector.tensor_tensor(out=tmp32b[:], in0=W4pair[:, :, 0],
                                        in1=W4pair[:, :, 1], op=OP.add)
                CW4 = sm.tile([128, 8], BF, tag="CW4", name=f"CW4_{c}")
                nc.vector.tensor_scalar(out=CW4[:], in0=tmp32b[:], scalar1=0.125,
                                        scalar2=None, op0=OP.mult)
                W3 = sm.tile([128, 16], BF, tag="W3", name=f"W3_{c}")
                nc.vector.tensor_scalar(out=W3[:], in0=W[:, 32:48], scalar1=1.0 / 6.0,
                                        scalar2=None, op0=OP.mult)
                # rows: reshape DMAs to DRAM staging; b3 u-grid rows via w3row
                nc.sync.dma_start(out=w1erow_d[0:1, t0:t1], in_=W1e[:])
                nc.sync.dma_start(out=w1orow_d[0:1, t0:t1], in_=W1o[:])
                nc.sync.dma_start(out=cw2row_d[0:1, t0:t1], in_=CW2[:])
                nc.sync.dma_start(out=cw4row_d[0:1, t0:t1], in_=CW4[:])
                nc.sync.dma_start(out=w3row[0:1, l0:l0 + 2048], in_=W3[:])
                ua0 = _ceil_div(t0, 3)
                na = _ceil_div(t1, 3) - ua0
                ub0 = _ceil_div(t0 - 1, 3)
                nb = _ceil_div(t1 - 1, 3) - ub0
                ud0 = _ceil_div(t0 - 2, 3)
                nd = _ceil_div(t1 - 2, 3) - ud0
                cwa = rws.tile([1, 344], BF, tag="cwa", name=f"cwa_{c}")
                nc.vector.tensor_tensor(
                    out=cwa[0:1, 0:na],
                    in0=w3row[0:1, 6 * ua0:6 * ua0 + 6 * na:6],
                    in1=w3row[0:1, 6 * ua0 + 1:6 * ua0 + 1 + 6 * na:6], op=OP.add)
                cwb = rws.tile([1, 344], BF, tag="cwb", name=f"cwb_{c}")
                nc.vector.tensor_copy(
                    out=cwb[0:1, 0:nb],
                    in_=w3row[0:1, 6 * ub0 + 2:6 * ub0 + 2 + 6 * nb:6])
                cwc = rws.tile([1, 344], BF, tag="cwc", name=f"cwc_{c}")
                nc.vector.tensor_copy(
                    out=cwc[0:1, 0:nb],
                    in_=w3row[0:1, 6 * ub0 + 3:6 * ub0 + 3 + 6 * nb:6])
                cwd = rws.tile([1, 344], BF, tag="cwd", name=f"cwd_{c}")
                nc.vector.tensor_tensor(
                    out=cwd[0:1, 0:nd],
                    in0=w3row[0:1, 6 * ud0 + 4:6 * ud0 + 4 + 6 * nd:6],
                    in1=w3row[0:1, 6 * ud0 + 5:6 * ud0 + 5 + 6 * nd:6], op=OP.add)
                nc.sync.dma_start(out=cwarow_d[0:1, ua0:ua0 + na], in_=cwa[0:1, 0:na])
                nc.sync.dma_start(out=cwbrow_d[0:1, ub0:ub0 + nb], in_=cwb[0:1, 0:nb])
                nc.sync.dma_start(out=cwcrow_d[0:1, ub0:ub0 + nb], in_=cwc[0:1, 0:nb])
                nc.sync.dma_start(out=cwdrow_d[0:1, ud0:ud0 + nd], in_=cwd[0:1, 0:nd])

                # ---- combine + transpose + store for chunk c
                s = c
                # u-grid windows for the three b=3 residue classes
                tA0 = t0 + (-t0) % 3
                nA = len(range(tA0, t0 + TSEG, 3))
                uA0 = tA0 // 3
                tB0 = t0 + (1 - t0) % 3
                nB = len(range(tB0, t0 + TSEG, 3))
                uB0 = (tB0 - 1) // 3
                tD0 = t0 + (2 - t0) % 3
                nD = len(range(tD0, t0 + TSEG, 3))
                uD0 = (tD0 - 2) // 3
                jbase = min(2 * uA0, 2 * uB0, 2 * uD0 + 1)
                jend = max(2 * (uA0 + nA - 1), 2 * (uB0 + nB - 1) + 1,
                           2 * (uD0 + nD - 1) + 1)
                nJ = jend - jbase + 1

                w1e_s = wsg.tile([128, TSEG], BF, tag="w1e", name=f"w1e_{s}")
                nc.sync.dma_start(
                    out=w1e_s[:],
                    in_=w1erow_d[0:1, t0:t0 + TSEG].partition_broadcast(128))
                w1o_s = wsg.tile([128, TSEG], BF, tag="w1o", name=f"w1o_{s}")
                nc.sync.dma_start(
                    out=w1o_s[:],
                    in_=w1orow_d[0:1, t0:t0 + TSEG].partition_broadcast(128))
                cw2_s = wsg.tile([128, TSEG], BF, tag="cw2", name=f"cw2_{s}")
                nc.sync.dma_start(
                    out=cw2_s[:],
                    in_=cw2row_d[0:1, t0:t0 + TSEG].partition_broadcast(128))
                cw4_s = wsg.tile([128, TSEG], BF, tag="cw4", name=f"cw4_{s}")
                nc.sync.dma_start(
                    out=cw4_s[:],
                    in_=cw4row_d[0:1, t0:t0 + TSEG].partition_broadcast(128))
                cwa_s = wsg.tile([128, 342], BF, tag="cwa", name=f"cwa_s{s}")
                nc.sync.dma_start(
                    out=cwa_s[:, 0:nA],
                    in_=cwarow_d[0:1, uA0:uA0 + nA].partition_broadcast(128))
                cwb_s = wsg.tile([128, 342], BF, tag="cwb", name=f"cwb_s{s}")
                nc.sync.dma_start(
                    out=cwb_s[:, 0:nB],
                    in_=cwbrow_d[0:1, uB0:uB0 + nB].partition_broadcast(128))
                cwc_s = wsg.tile([128, 342], BF, tag="cwc", name=f"cwc_s{s}")
                nc.sync.dma_start(
                    out=cwc_s[:, 0:nB],
                    in_=cwcrow_d[0:1, uB0:uB0 + nB].partition_broadcast(128))
                cwd_s = wsg.tile([128, 342], BF, tag="cwd", name=f"cwd_s{s}")
                nc.sync.dma_start(
                    out=cwd_s[:, 0:nD],
                    in_=cwdrow_d[0:1, uD0:uD0 + nD].partition_broadcast(128))

                for dc in range(NDC):
                    ypair = y[dc][:, 0:L].rearrange("p (t two) -> p t two", two=2)
                    # per-seg pooled blocks (GPSIMD)
                    p2_s = sgp.tile([128, TSEG], BF, tag="p2s", name=f"p2s_{s}_{dc}")
                    nc.gpsimd.tensor_tensor(out=p2_s[:],
                                            in0=ypair[:, t0:t0 + TSEG, 0],
                                            in1=ypair[:, t0:t0 + TSEG, 1], op=OP.add)
                    p4_s = sgp.tile([128, TSEG // 2], BF, tag="p4s",
                                    name=f"p4s_{s}_{dc}")
                    p2sp = p2_s[:].rearrange("p (v two) -> p v two", two=2)
                    nc.gpsimd.tensor_tensor(out=p4_s[:], in0=p2sp[:, :, 0],
                                            in1=p2sp[:, :, 1], op=OP.add)
                    p3_s = sgp.tile([128, 688], BF, tag="p3s", name=f"p3s_{s}_{dc}")
                    nc.gpsimd.tensor_tensor(
                        out=p3_s[:, 0:nJ],
                        in0=y[dc][:, 3 * jbase:3 * (jbase + nJ) - 2:3],
                        in1=y[dc][:, 3 * jbase + 1:3 * (jbase + nJ) - 1:3], op=OP.add)
                    nc.gpsimd.tensor_tensor(
                        out=p3_s[:, 0:nJ], in0=p3_s[:, 0:nJ],
                        in1=y[dc][:, 3 * jbase + 2:3 * (jbase + nJ):3], op=OP.add)

                    m1e = ctp.tile([128, TSEG], BF, tag="ct", name=f"m1e_{s}_{dc}")
                    nc.vector.tensor_tensor(out=m1e[:], in0=ypair[:, t0:t0 + TSEG, 0],
                                            in1=w1e_s[:], op=OP.mult)
                    m1o = ctp.tile([128, TSEG], BF, tag="ct", name=f"m1o_{s}_{dc}")
                    nc.vector.tensor_tensor(out=m1o[:], in0=ypair[:, t0:t0 + TSEG, 1],
                                            in1=w1o_s[:], op=OP.mult)
                    a12 = ctp.tile([128, TSEG], BF, tag="ct", name=f"a12_{s}_{dc}")
                    nc.vector.tensor_tensor(out=a12[:], in0=m1e[:], in1=m1o[:],
                                            op=OP.add)
                    m2 = ctp.tile([128, TSEG], BF, tag="ct", name=f"m2_{s}_{dc}")
                    nc.vector.tensor_tensor(out=m2[:], in0=p2_s[:],
                                            in1=cw2_s[:], op=OP.mult)
                    m4 = ctp.tile([128, TSEG], BF, tag="ct", name=f"m4_{s}_{dc}")
                    nc.vector.tensor_tensor(
                        out=m4[:],
                        in0=p4_s[:].unsqueeze(2).to_broadcast([128, TSEG // 2, 2]),
                        in1=cw4_s[:], op=OP.mult)
                    acc = accp.tile([128, TSEG], BF, tag="acc", name=f"acc_{s}_{dc}")
                    nc.vector.tensor_tensor(out=acc[:], in0=a12[:], in1=m2[:],
                                            op=OP.add)
                    nc.vector.tensor_tensor(out=acc[:], in0=acc[:], in1=m4[:],
                                            op=OP.add)
                    # b=3 terms on their u-grids
                    for bi, (n_u, u0, coff, poff, cw_s) in enumerate((
                            (nA, uA0, tA0 - t0, 0, cwa_s),
                            (nB, uB0, tB0 - t0, 0, cwb_s),
                            (nB, uB0, tB0 - t0, 1, cwc_s),
                            (nD, uD0, tD0 - t0, 1, cwd_s))):
                        tb3 = btp.tile([128, 342], BF, tag="bt",
                                       name=f"tb3_{s}_{dc}_{bi}")
                        j0b = 2 * u0 + poff - jbase
                        nc.vector.tensor_tensor(
                            out=tb3[:, 0:n_u],
                            in0=p3_s[:, j0b:j0b + 2 * n_u - 1:2],
                            in1=cw_s[:, 0:n_u], op=OP.mult)
                        accv = acc[:, coff:TSEG:3]
                        nc.vector.tensor_tensor(out=accv[:, 0:n_u], in0=accv[:, 0:n_u],
                                                in1=tb3[:, 0:n_u], op=OP.add)
                    # transpose [d, t] -> [t, d] and store
                    for q4 in range(2):
                        pt = psT.tile([128, 512], BF, tag="tp",
                                      name=f"pt_{s}_{dc}_{q4}")
                        for q in range(4):
                            nc.tensor.transpose(
                                out=pt[:, q * 128:(q + 1) * 128],
                                in_=acc[:, (q4 * 4 + q) * 128:(q4 * 4 + q + 1) * 128],
                                identity=ident_t[:])
                        ot = otp.tile([128, 512], F32, tag="ot",
                                      name=f"ot_{s}_{dc}_{q4}")
                        nc.scalar.copy(out=ot[:], in_=pt[:])
                        tb0 = s * 8 + q4 * 4
                        nc.sync.dma_start(
                            out=ov[:, tb0:tb0 + 4, dc, :],
                            in_=ot[:].rearrange("p (tb c) -> p tb c", c=128))
    nc.compile()
    return nc


def _get_nc():
    global _NC
    if _NC is None:
        _NC = _build()
    return _NC


def kernel(input_ids, emb, conv_w, conv_b, score_w):
    global LAST_RESULT
    nc = _get_nc()
    input_ids = np.asarray(input_ids)
    emb = np.asarray(emb, dtype=np.float32)
    conv_w = np.asarray(conv_w, dtype=np.float32)
    conv_b = np.asarray(conv_b, dtype=np.float32)
    score_w = np.asarray(score_w, dtype=np.float32)
    B = input_ids.shape[0]

    G = np.einsum("oik,vi->kvo", conv_w.astype(np.float64),
                  emb.astype(np.float64)).astype(np.float32)  # [K, V, D]
    gws = np.zeros((128, 40, 128), np.float32)
    for k in range(K):
        for vc in range(NVC):
            for dc in range(NDC):
                gws[:, (k * 2 + vc) * 4 + dc, :] = \
                    G[k, vc * 128:(vc + 1) * 128, dc * 128:(dc + 1) * 128]
    gws = gws.reshape(128, 40 * 128).astype(bf16)
    iot = np.stack([np.arange(128), np.arange(128) + 128], axis=1).astype(np.float32)
    scw = score_w.reshape(4, 128).T.astype(bf16)
    biasm = conv_b.reshape(4, 128).T.astype(np.float32)
    ident = np.eye(128, dtype=np.float32).astype(bf16)
    idsb = input_ids.astype(np.float32).astype(bf16)

    in_maps = [{"ids": np.ascontiguousarray(idsb[c:c + 1]), "gws": gws, "iot": iot,
                "scw": scw, "bias": biasm, "ident": ident} for c in range(B)]
    res = run_bass_kernel_spmd(nc, in_maps, core_ids=list(range(B)), trace=TRACE)
    LAST_RESULT = res
    return np.stack([res.results[c]["out"] for c in range(B)]).astype(np.float32)
